# revision 25
# baseline (speedup 1.0000x reference)
"""BinaryConv2d (3x3, SAME, NHWC) on 8 trn2 NeuronCores.

Sharding: data-parallel over batch — 2 images per core; the tiny binarized
weight tensor is replicated. Per core, the two images are packed on the two
64-partition halves of SBUF so matmuls run concurrently on disjoint
row-groups of the 128x128 PE array.

Math: fp8(e4m3) DoubleRow matmuls at 2x bf16 rate. x is decomposed as
x = hi + lo (both e4m3); the 9 conv taps become 14 virtual taps (9 hi + 5
lo corrections on taps 4-8) = exactly 7 DoubleRow pairs per image per
512-position slot, vs 9 bf16 matmuls before (7/9 tensor time). Taps 0-3
run hi-only; the resulting rel err is 2.65e-2 * sqrt(4/9) ~= 1.78e-2,
under the 2e-2 gate (verified in fp64 against the jax reference).

Layout trick: the conv is evaluated on a flat q-grid over a zero-padded
226-wide plane, so each of the 9 taps is a pure free-dim offset
(dh*226 + dw) into the same SBUF x tile; row-crossing outputs land in 2
garbage columns per row that the host discards. The hi and lo planes live
in one SBUF tile [128, 2, COLS]; a DoubleRow rhs is a hand-built
[64, 2, n] access pattern whose middle-dim stride selects either a
second hi tap (intra-plane) or the same tap's lo plane.
"""

import sys

for _p in ("/opt/trn_rl_repo",):
    if _p not in sys.path:
        sys.path.insert(0, _p)

import bass_rust
import ml_dtypes
import numpy as np

BF16 = ml_dtypes.bfloat16
E4M3 = ml_dtypes.float8_e4m3

N_CORES = 8
IMG_PER_CORE = 2
H = W_IMG = 224
C_IN, C_OUT = 64, 128
PR, PC = 227, 226  # padded plane: 226 rows of data + 1 extra zero row
PLANE = PR * PC  # 51302
QOUT = H * PC  # 50624 q-positions per image (2 garbage cols per row)
SLOT = 512  # q-positions per matmul (one PSUM bank of fp32)
N_SLOTS = (QOUT + SLOT - 1) // SLOT  # 99 (last slot = 448)
SLOTS_PER_CHUNK = 16
HALO = 2 * PC + 2  # 454: max tap offset
CHUNK_Q = SLOTS_PER_CHUNK * SLOT
XTILE_COLS = CHUNK_Q + HALO
STAGE_SLOTS = 8
STAGE_Q = STAGE_SLOTS * SLOT

# tap t (row-major over the 3x3 kernel) offset into the flat padded plane
TAP_OFF = [(t // 3) * PC + (t % 3) for t in range(9)]
# 7 DoubleRow pairs: (tapA, tapB, planeB). planeA is always 0 (hi).
# Taps 0-3 are hi-only (paired hi-hi); taps 4-8 get the lo correction
# (paired with their own lo plane).
PAIRS = [
    (0, 1, 0),
    (2, 3, 0),
    (4, 4, 1),
    (5, 5, 1),
    (6, 6, 1),
    (7, 7, 1),
    (8, 8, 1),
]
NPAIR = len(PAIRS)


def _chunk_plan():
    """(start_slot -> n_slots): geometric ramp so early chunks land
    just-in-time, then steady 16-slot chunks."""
    plan = {}
    s, size = 0, 1
    while s < N_SLOTS:
        n = min(size, N_SLOTS - s, SLOTS_PER_CHUNK)
        plan[s] = n
        s += n
        size *= 2
    return plan


def _stage_plan():
    """(start_slot -> n_slots): 8-slot stages, with a small split tail so
    the final out-DMA after the last matmul is tiny."""
    plan = {}
    s = 0
    while s < N_SLOTS:
        rem = N_SLOTS - s
        if rem >= STAGE_SLOTS:
            n = STAGE_SLOTS
        elif rem == 3:
            n = 2
        else:
            n = rem if rem <= 2 else rem - 1
        plan[s] = n
        s += n
    return plan


_COMPILED = None
_LAST_RES = None


def _ld_key(inst):
    ap = inst.ins[0]
    return (
        str(ap.memref),
        ap.offset,
        tuple(tuple(x) for x in ap.ap),
        str(ap.dtype),
        str(inst.perf_mode),
        str(inst.is_transpose),
        tuple(inst.tile_position or (0, 0)),
    )


def _dedup_ldweights(nc, mybir):
    """Remove InstLdweights that reload the exact weights already resident in
    the same PE tile (same weights AP + perf_mode + tile_position as the
    previous load for that tile). The PE keeps the stationary tensor across
    matmuls, so consecutive same-weight matmuls only need one load. Any sync
    carried by a removed load is transferred to the next matmul."""
    n_removed = 0
    for f in nc.m.functions:
        remap = {}
        for b in f.blocks:
            last = {}
            new_list = []
            pending = None
            for inst in list(b.instructions):
                tn = type(inst).__name__
                if tn == "InstLdweights":
                    key = _ld_key(inst)
                    tp = tuple(inst.tile_position or (0, 0))
                    if last.get(tp) == key:
                        si = inst.sync_info
                        pending = (
                            list(si.on_wait) if si else [],
                            list(si.on_update) if si else [],
                            inst.name,
                            pending,
                        )
                        n_removed += 1
                        continue
                    last[tp] = key
                elif tn == "InstMatmult":
                    while pending is not None:
                        w, u, oldname, pending = pending
                        si = inst.sync_info or mybir.SyncInfo(on_wait=[], on_update=[])
                        si.on_wait = list(si.on_wait) + w
                        si.on_update = list(si.on_update) + u
                        inst.sync_info = si
                        remap[oldname] = inst.name
                new_list.append(inst)
            b.instructions[:] = new_list
        if remap:
            for b in f.blocks:
                for inst in b.instructions:
                    inst.remap_dependency_names(remap)
    return n_removed


def _build():
    import concourse.mybir as mybir
    import concourse.tile as tile
    from concourse import bacc

    nc = bacc.Bacc(
        "TRN2", target_bir_lowering=False, debug=False, num_devices=N_CORES
    )
    x_d = nc.dram_tensor("x", [128, 2, PLANE], mybir.dt.float8e4, kind="ExternalInput")
    w_d = nc.dram_tensor(
        "w", [128, NPAIR * 2 * 128], mybir.dt.float8e4, kind="ExternalInput"
    )
    b_d = nc.dram_tensor("b", [128, 1], mybir.dt.float32, kind="ExternalInput")
    o_d = nc.dram_tensor(
        "out", [128, IMG_PER_CORE * QOUT], mybir.dt.bfloat16, kind="ExternalOutput"
    )

    ident = mybir.ActivationFunctionType.Identity
    dr = mybir.MatmulPerfMode.DoubleRow

    with tile.TileContext(nc) as tc:
        with (
            tc.tile_pool(name="sbuf", bufs=1) as cpool,
            tc.tile_pool(name="psum", bufs=2, space="PSUM") as ppool,
        ):
            xpool = spool = cpool
            # Critical-path-first ordering on the HWDGE ring: pair-0 weights,
            # first small x chunk, rest of the weights, bias, then the
            # geometrically ramped x chunks.
            w_sb = cpool.tile([128, NPAIR * 2 * 128], mybir.dt.float8e4, tag="w")
            nc.sync.dma_start(w_sb[:, 0:256], w_d[:, 0:256])
            b_sb = cpool.tile([128, 1], mybir.dt.float32, tag="b")

            # One HAM activity window (~3.4us) of dummy cold matmuls on a
            # zeroed tile, sized to finish as the first x chunk lands: the
            # PE clock-gate releases before the real stream starts, so it
            # runs at 2.4GHz from matmul 0 (results are never read).
            warm_src = cpool.tile([128, SLOT], mybir.dt.bfloat16, tag="warm")
            nc.vector.memset(warm_src[:], 0.0)
            # reuse psa0's bank rotation: warm occupies one generation, long
            # done before the second rotation returns to this bank
            warm_ps = ppool.tile([128, SLOT], mybir.dt.float32, tag="psa0")
            N_WARM = 10
            for i in range(N_WARM):
                nc.tensor.matmul(
                    warm_ps[:, :],
                    lhsT=warm_src[:, 0:128],
                    rhs=warm_src[:, :],
                    start=(i == 0),
                    stop=(i == N_WARM - 1),
                )

            chunk_plan = _chunk_plan()
            stage_plan = _stage_plan()
            xt = None
            st_a = st_b = None
            stage_end = -1
            s = 0
            while s < N_SLOTS:
                # group of up to 2 slots sharing each pair's weight load
                gslots = []
                for si in (s, s + 1):
                    if si < N_SLOTS and (si == s or si not in chunk_plan):
                        gslots.append(si)

                if s in chunk_plan:
                    cq0 = s * SLOT
                    ext = min(QOUT, cq0 + chunk_plan[s] * SLOT) - cq0 + HALO
                    xt = xpool.tile(
                        [128, 2, XTILE_COLS], mybir.dt.float8e4, tag="x", bufs=4
                    )
                    nc.sync.dma_start(xt[:, :, :ext], x_d[:, :, cq0 : cq0 + ext])
                    if s == 0:
                        nc.sync.dma_start(w_sb[:, 256:], w_d[:, 256:])
                        nc.sync.dma_start(b_sb[:], b_d[:])

                ps_tiles = [
                    (
                        ppool.tile(
                            [128, SLOT], mybir.dt.float32, tag=f"psa{j}", name=f"psa{j}"
                        ),
                        ppool.tile(
                            [128, SLOT], mybir.dt.float32, tag=f"psb{j}", name=f"psb{j}"
                        ),
                    )
                    for j in range(len(gslots))
                ]

                for p, (ta, tb, plb) in enumerate(PAIRS):
                    for half in (0, 1):
                        p0 = 64 * half
                        lhsT = w_sb[
                            p0 : p0 + 64, p * 256 : (p + 1) * 256
                        ].rearrange("p (a b) -> p a b", a=2)
                        for j, si in enumerate(gslots):
                            q0 = si * SLOT
                            n = min(SLOT, QOUT - q0)
                            oa = q0 - cq0 + TAP_OFF[ta]
                            ob = q0 - cq0 + TAP_OFF[tb]
                            s1 = plb * XTILE_COLS + ob - oa
                            rhs = xt[p0 : p0 + 64, 0, oa : oa + n].copy()
                            rhs.ap = bass_rust.VecI64Pair(
                                [tuple(rhs.ap[0]), (s1, 2), (1, n)]
                            )
                            nc.tensor.matmul(
                                ps_tiles[j][half][:, :n],
                                lhsT=lhsT,
                                rhs=rhs,
                                start=(p == 0),
                                stop=(p == NPAIR - 1),
                                perf_mode=dr,
                            )

                for j, si in enumerate(gslots):
                    q0 = si * SLOT
                    n = min(SLOT, QOUT - q0)
                    if si in stage_plan:
                        g0 = q0
                        gext = min(QOUT, g0 + stage_plan[si] * SLOT) - g0
                        stage_end = si + stage_plan[si] - 1
                        st_a = spool.tile(
                            [128, STAGE_Q], mybir.dt.bfloat16, tag="sa", bufs=3
                        )
                        st_b = spool.tile(
                            [128, STAGE_Q], mybir.dt.bfloat16, tag="sb", bufs=3
                        )
                    so = q0 - g0
                    nc.vector.tensor_scalar_add(
                        st_a[:, so : so + n], ps_tiles[j][0][:, :n], b_sb[:]
                    )
                    nc.scalar.activation(
                        st_b[:, so : so + n], ps_tiles[j][1][:, :n], ident, bias=b_sb[:]
                    )
                    if si == stage_end:
                        nc.sync.dma_start(o_d[:, g0 : g0 + gext], st_a[:, :gext])
                        nc.sync.dma_start(
                            o_d[:, QOUT + g0 : QOUT + g0 + gext], st_b[:, :gext]
                        )

                s += len(gslots)

    _dedup_ldweights(nc, mybir)
    nc.compile()
    return nc


def _get_nc():
    global _COMPILED
    if _COMPILED is None:
        _COMPILED = _build()
    return _COMPILED


def kernel(x: np.ndarray, W: np.ndarray, b: np.ndarray) -> np.ndarray:
    from concourse.bass_utils import run_bass_kernel_spmd

    nc = _get_nc()

    xf = np.asarray(x, dtype=np.float32)
    hi = xf.astype(E4M3)
    lo = (xf - hi.astype(np.float32)).astype(E4M3)
    X = np.zeros((N_CORES, IMG_PER_CORE, C_IN, 2, PR, PC), E4M3)
    for pl, arr in ((0, hi), (1, lo)):
        X[:, :, :, pl, 1 : H + 1, 1 : W_IMG + 1] = arr.reshape(
            N_CORES, IMG_PER_CORE, H, W_IMG, C_IN
        ).transpose(0, 1, 4, 2, 3)
    Xf = X.reshape(N_CORES, 128, 2, PLANE)

    Wb = np.sign(np.asarray(W, dtype=np.float32)).reshape(9, C_IN, C_OUT)
    wh = np.empty((2, C_IN, NPAIR, 2, C_OUT), np.float32)
    for p, (ta, tb, _plb) in enumerate(PAIRS):
        wh[:, :, p, 0] = Wb[ta][None]
        wh[:, :, p, 1] = Wb[tb][None]
    wh = np.ascontiguousarray(
        wh.astype(E4M3).reshape(128, NPAIR * 2 * C_OUT)
    )

    bh = np.ascontiguousarray(np.asarray(b, dtype=np.float32).reshape(128, 1))

    in_maps = [{"x": Xf[c], "w": wh, "b": bh} for c in range(N_CORES)]
    res = run_bass_kernel_spmd(nc, in_maps, list(range(N_CORES)))
    global _LAST_RES
    _LAST_RES = res

    O = np.stack([res.results[c]["out"] for c in range(N_CORES)])
    O = O.reshape(N_CORES, C_OUT, IMG_PER_CORE, H, PC)[:, :, :, :, :W_IMG]
    y = O.transpose(0, 2, 3, 4, 1).reshape(16, H, W_IMG, C_OUT)
    return np.ascontiguousarray(y).astype(np.float32)


# revision 29
# speedup vs baseline: 1.0095x; 1.0095x over previous
"""BinaryConv2d (3x3, SAME, NHWC) on 8 trn2 NeuronCores.

Sharding: data-parallel over batch — 2 images per core; the tiny binarized
weight tensor is replicated. Per core, the two images are packed on the two
64-partition halves of SBUF so matmuls run concurrently on disjoint
row-groups of the 128x128 PE array.

Math: fp8(e4m3) DoubleRow matmuls at 2x bf16 rate. x is decomposed as
x = hi + lo (both e4m3); the 9 conv taps become 14 virtual taps (9 hi + 5
lo corrections on taps 4-8) = exactly 7 DoubleRow pairs per image per
512-position slot, vs 9 bf16 matmuls before (7/9 tensor time). Taps 0-3
run hi-only; the resulting rel err is 2.65e-2 * sqrt(4/9) ~= 1.78e-2,
under the 2e-2 gate (verified in fp64 against the jax reference).

Layout trick: the conv is evaluated on a flat q-grid over a zero-padded
226-wide plane, so each of the 9 taps is a pure free-dim offset
(dh*226 + dw) into the same SBUF x tile; row-crossing outputs land in 2
garbage columns per row that the host discards. The hi and lo planes live
in one SBUF tile [128, 2, COLS]; a DoubleRow rhs is a hand-built
[64, 2, n] access pattern whose middle-dim stride selects either a
second hi tap (intra-plane) or the same tap's lo plane.
"""

import sys

for _p in ("/opt/trn_rl_repo",):
    if _p not in sys.path:
        sys.path.insert(0, _p)

import bass_rust
import ml_dtypes
import numpy as np

BF16 = ml_dtypes.bfloat16
E4M3 = ml_dtypes.float8_e4m3

N_CORES = 8
IMG_PER_CORE = 2
H = W_IMG = 224
C_IN, C_OUT = 64, 128
PR, PC = 227, 226  # padded plane: 226 rows of data + 1 extra zero row
PLANE = PR * PC  # 51302
QOUT = H * PC  # 50624 q-positions per image (2 garbage cols per row)
SLOT = 512  # q-positions per matmul (one PSUM bank of fp32)
N_SLOTS = (QOUT + SLOT - 1) // SLOT  # 99 (last slot = 448)
SLOTS_PER_CHUNK = 16
HALO = 2 * PC + 2  # 454: max tap offset
CHUNK_Q = SLOTS_PER_CHUNK * SLOT
XTILE_COLS = CHUNK_Q + HALO
STAGE_SLOTS = 8
STAGE_Q = STAGE_SLOTS * SLOT

# tap t (row-major over the 3x3 kernel) offset into the flat padded plane
TAP_OFF = [(t // 3) * PC + (t % 3) for t in range(9)]
# 7 DoubleRow pairs: (tapA, tapB, planeB). planeA is always 0 (hi).
# Taps 0-3 are hi-only (paired hi-hi); taps 4-8 get the lo correction
# (paired with their own lo plane).
PAIRS = [
    (0, 1, 0),
    (2, 3, 0),
    (4, 4, 1),
    (5, 5, 1),
    (6, 6, 1),
    (7, 7, 1),
    (8, 8, 1),
]
NPAIR = len(PAIRS)


def _chunk_plan():
    """(start_slot -> n_slots): geometric ramp so early chunks land
    just-in-time, then steady 16-slot chunks."""
    plan = {}
    s, size = 0, 1
    while s < N_SLOTS:
        n = min(size, N_SLOTS - s, SLOTS_PER_CHUNK)
        plan[s] = n
        s += n
        size *= 2
    return plan


def _stage_plan():
    """(start_slot -> n_slots): 8-slot stages, with a small split tail so
    the final out-DMA after the last matmul is tiny."""
    plan = {}
    s = 0
    while s < N_SLOTS:
        rem = N_SLOTS - s
        if rem >= STAGE_SLOTS:
            n = STAGE_SLOTS
        elif rem == 3:
            n = 2
        else:
            n = rem if rem <= 2 else rem - 1
        plan[s] = n
        s += n
    return plan


_COMPILED = None
_LAST_RES = None


def _ld_key(inst):
    ap = inst.ins[0]
    return (
        str(ap.memref),
        ap.offset,
        tuple(tuple(x) for x in ap.ap),
        str(ap.dtype),
        str(inst.perf_mode),
        str(inst.is_transpose),
        tuple(inst.tile_position or (0, 0)),
    )


def _dedup_ldweights(nc, mybir):
    """Remove InstLdweights that reload the exact weights already resident in
    the same PE tile (same weights AP + perf_mode + tile_position as the
    previous load for that tile). The PE keeps the stationary tensor across
    matmuls, so consecutive same-weight matmuls only need one load. Any sync
    carried by a removed load is transferred to the next matmul."""
    n_removed = 0
    for f in nc.m.functions:
        remap = {}
        for b in f.blocks:
            last = {}
            new_list = []
            pending = None
            for inst in list(b.instructions):
                tn = type(inst).__name__
                if tn == "InstLdweights":
                    key = _ld_key(inst)
                    tp = tuple(inst.tile_position or (0, 0))
                    if last.get(tp) == key:
                        si = inst.sync_info
                        pending = (
                            list(si.on_wait) if si else [],
                            list(si.on_update) if si else [],
                            inst.name,
                            pending,
                        )
                        n_removed += 1
                        continue
                    last[tp] = key
                elif tn == "InstMatmult":
                    while pending is not None:
                        w, u, oldname, pending = pending
                        si = inst.sync_info or mybir.SyncInfo(on_wait=[], on_update=[])
                        si.on_wait = list(si.on_wait) + w
                        si.on_update = list(si.on_update) + u
                        inst.sync_info = si
                        remap[oldname] = inst.name
                new_list.append(inst)
            b.instructions[:] = new_list
        if remap:
            for b in f.blocks:
                for inst in b.instructions:
                    inst.remap_dependency_names(remap)
    return n_removed


def _build():
    import concourse.mybir as mybir
    import concourse.tile as tile
    from concourse import bacc

    nc = bacc.Bacc(
        "TRN2", target_bir_lowering=False, debug=False, num_devices=N_CORES
    )
    x_d = nc.dram_tensor("x", [128, 2, PLANE], mybir.dt.float8e4, kind="ExternalInput")
    w_d = nc.dram_tensor(
        "w", [128, NPAIR * 2 * 128], mybir.dt.float8e4, kind="ExternalInput"
    )
    b_d = nc.dram_tensor("b", [128, 1], mybir.dt.float32, kind="ExternalInput")
    o_d = nc.dram_tensor(
        "out", [128, IMG_PER_CORE * QOUT], mybir.dt.bfloat16, kind="ExternalOutput"
    )

    ident = mybir.ActivationFunctionType.Identity
    dr = mybir.MatmulPerfMode.DoubleRow

    with tile.TileContext(nc) as tc:
        with (
            tc.tile_pool(name="sbuf", bufs=1) as cpool,
            tc.tile_pool(name="psum", bufs=2, space="PSUM") as ppool,
        ):
            xpool = spool = cpool
            # Critical-path-first ordering on the HWDGE ring: pair-0 weights,
            # first small x chunk, rest of the weights, bias, then the
            # geometrically ramped x chunks.
            w_sb = cpool.tile([128, NPAIR * 2 * 128], mybir.dt.float8e4, tag="w")
            nc.sync.dma_start(w_sb[:, 0:256], w_d[:, 0:256])
            b_sb = cpool.tile([128, 1], mybir.dt.float32, tag="b")

            # One HAM activity window (~3.4us) of dummy cold matmuls on a
            # zeroed tile, sized to finish as the first x chunk lands: the
            # PE clock-gate releases before the real stream starts, so it
            # runs at 2.4GHz from matmul 0 (results are never read).
            warm_src = cpool.tile([128, SLOT], mybir.dt.bfloat16, tag="warm")
            nc.vector.memset(warm_src[:], 0.0)
            # reuse psa0's bank rotation: warm occupies one generation, long
            # done before the second rotation returns to this bank
            warm_ps = ppool.tile([128, SLOT], mybir.dt.float32, tag="psa0")
            N_WARM = 10
            for i in range(N_WARM):
                nc.tensor.matmul(
                    warm_ps[:, :],
                    lhsT=warm_src[:, 0:128],
                    rhs=warm_src[:, :],
                    start=(i == 0),
                    stop=(i == N_WARM - 1),
                )

            chunk_plan = _chunk_plan()
            stage_plan = _stage_plan()
            xt = None
            st_a = st_b = None
            stage_end = -1
            s = 0
            snake = False
            while s < N_SLOTS:
                # group of up to 2 slots sharing each pair's weight load
                gslots = []
                for si in (s, s + 1):
                    if si < N_SLOTS and (si == s or si not in chunk_plan):
                        gslots.append(si)

                if s in chunk_plan:
                    cq0 = s * SLOT
                    ext = min(QOUT, cq0 + chunk_plan[s] * SLOT) - cq0 + HALO
                    xt = xpool.tile(
                        [128, 2, XTILE_COLS], mybir.dt.float8e4, tag="x", bufs=4
                    )
                    nc.sync.dma_start(xt[:, :, :ext], x_d[:, :, cq0 : cq0 + ext])
                    if s == 0:
                        nc.sync.dma_start(w_sb[:, 256:], w_d[:, 256:])
                        nc.sync.dma_start(b_sb[:], b_d[:])

                ps_tiles = [
                    (
                        ppool.tile(
                            [128, SLOT], mybir.dt.float32, tag=f"psa{j}", name=f"psa{j}"
                        ),
                        ppool.tile(
                            [128, SLOT], mybir.dt.float32, tag=f"psb{j}", name=f"psb{j}"
                        ),
                    )
                    for j in range(len(gslots))
                ]

                for idx, (p, (ta, tb, plb)) in enumerate(list(enumerate(PAIRS))):
                    for half in (0, 1):
                        p0 = 64 * half
                        lhsT = w_sb[
                            p0 : p0 + 64, p * 256 : (p + 1) * 256
                        ].rearrange("p (a b) -> p a b", a=2)
                        for j, si in enumerate(gslots):
                            q0 = si * SLOT
                            n = min(SLOT, QOUT - q0)
                            oa = q0 - cq0 + TAP_OFF[ta]
                            ob = q0 - cq0 + TAP_OFF[tb]
                            s1 = plb * XTILE_COLS + ob - oa
                            rhs = xt[p0 : p0 + 64, 0, oa : oa + n].copy()
                            rhs.ap = bass_rust.VecI64Pair(
                                [tuple(rhs.ap[0]), (s1, 2), (1, n)]
                            )
                            nc.tensor.matmul(
                                ps_tiles[j][half][:, :n],
                                lhsT=lhsT,
                                rhs=rhs,
                                start=(idx == 0),
                                stop=(idx == NPAIR - 1),
                                perf_mode=dr,
                            )

                for j, si in enumerate(gslots):
                    q0 = si * SLOT
                    n = min(SLOT, QOUT - q0)
                    if si in stage_plan:
                        g0 = q0
                        gext = min(QOUT, g0 + stage_plan[si] * SLOT) - g0
                        stage_end = si + stage_plan[si] - 1
                        st_a = spool.tile(
                            [128, STAGE_Q], mybir.dt.bfloat16, tag="sa", bufs=3
                        )
                        st_b = spool.tile(
                            [128, STAGE_Q], mybir.dt.bfloat16, tag="sb", bufs=3
                        )
                    so = q0 - g0
                    nc.vector.tensor_scalar_add(
                        st_a[:, so : so + n], ps_tiles[j][0][:, :n], b_sb[:]
                    )
                    nc.scalar.activation(
                        st_b[:, so : so + n], ps_tiles[j][1][:, :n], ident, bias=b_sb[:]
                    )
                    if si == stage_end:
                        nc.sync.dma_start(o_d[:, g0 : g0 + gext], st_a[:, :gext])
                        nc.sync.dma_start(
                            o_d[:, QOUT + g0 : QOUT + g0 + gext], st_b[:, :gext]
                        )

                s += len(gslots)

    _dedup_ldweights(nc, mybir)
    nc.compile()
    return nc


def _get_nc():
    global _COMPILED
    if _COMPILED is None:
        _COMPILED = _build()
    return _COMPILED


def kernel(x: np.ndarray, W: np.ndarray, b: np.ndarray) -> np.ndarray:
    from concourse.bass_utils import run_bass_kernel_spmd

    nc = _get_nc()

    xf = np.asarray(x, dtype=np.float32)
    hi = xf.astype(E4M3)
    lo = (xf - hi.astype(np.float32)).astype(E4M3)
    X = np.zeros((N_CORES, IMG_PER_CORE, C_IN, 2, PR, PC), E4M3)
    for pl, arr in ((0, hi), (1, lo)):
        X[:, :, :, pl, 1 : H + 1, 1 : W_IMG + 1] = arr.reshape(
            N_CORES, IMG_PER_CORE, H, W_IMG, C_IN
        ).transpose(0, 1, 4, 2, 3)
    Xf = X.reshape(N_CORES, 128, 2, PLANE)

    Wb = np.sign(np.asarray(W, dtype=np.float32)).reshape(9, C_IN, C_OUT)
    wh = np.empty((2, C_IN, NPAIR, 2, C_OUT), np.float32)
    for p, (ta, tb, _plb) in enumerate(PAIRS):
        wh[:, :, p, 0] = Wb[ta][None]
        wh[:, :, p, 1] = Wb[tb][None]
    wh = np.ascontiguousarray(
        wh.astype(E4M3).reshape(128, NPAIR * 2 * C_OUT)
    )

    bh = np.ascontiguousarray(np.asarray(b, dtype=np.float32).reshape(128, 1))

    in_maps = [{"x": Xf[c], "w": wh, "b": bh} for c in range(N_CORES)]
    res = run_bass_kernel_spmd(nc, in_maps, list(range(N_CORES)))
    global _LAST_RES
    _LAST_RES = res

    O = np.stack([res.results[c]["out"] for c in range(N_CORES)])
    O = O.reshape(N_CORES, C_OUT, IMG_PER_CORE, H, PC)[:, :, :, :, :W_IMG]
    y = O.transpose(0, 2, 3, 4, 1).reshape(16, H, W_IMG, C_OUT)
    return np.ascontiguousarray(y).astype(np.float32)
